# revision 9
# baseline (speedup 1.0000x reference)
"""Masked dot-product attention (B=16, Q=K=2048, D=512, fp32) on 8 TRN2 cores.

Data-parallel over batch: each of the 8 NeuronCores runs full [Q,K] attention
for 2 batches. Per batch, on-chip:

  S-pass :  S[q,k] tiles = Qt.T @ Kt   (f32r matmuls, PSUM [128,512] x4 banks,
            lhsT reused across the 4 k-blocks). DVE adds the additive key mask
            (-30000 on masked cols -> exp underflows to exact 0), ACT exp
            writes the W staging tile; accum_out yields row-sum partials free.
  expSt  :  PE-transposes of the exp(S) tiles (f32 -> PSUM) + DVE copy with
            f32r rounding build the [k,q]-layout operand for the context
            matmul -- cheaper than recomputing S^T (192ns vs 293ns/op).
  W      :  in-place ACT Copy with per-partition scale = 1/rowsum, DMA out.
  ctx    :  PSUM accum of expSt.T @ V over 16 k-tiles, DVE scale by 1/rowsum.

Q is prescaled by 1/sqrt(D) on the host; masks are data (program is
value-agnostic w.r.t. valid_lens).

`repeat=R` wraps the whole body in a hardware loop (timing harness only).
"""

import contextlib

import numpy as np

import concourse.bass as bass  # noqa: F401
import concourse.mybir as mybir
import concourse.tile as tile
from concourse import bacc
from concourse.bass_utils import run_bass_kernel_spmd

F32 = mybir.dt.float32
F32R = mybir.dt.float32r
BF16 = mybir.dt.bfloat16
AF = mybir.ActivationFunctionType
ALU = mybir.AluOpType

B, NQ, NK, ND = 16, 2048, 2048, 512
NCORES = 8
BPC = B // NCORES  # batches per core
MASK_NEG = -30000.0


def _make_identity(nc, ident):
    nc.gpsimd.memset(ident, 0.0)
    sq = ident.shape[0]
    nc.gpsimd.affine_select(
        out=ident,
        in_=ident,
        compare_op=ALU.not_equal,
        fill=1.0,
        base=0,
        pattern=[[-1, sq]],
        channel_multiplier=1,
    )


def build_nc(bpc=BPC, nq=NQ, nk=NK, nd=ND, ctx_bf16=False, repeat=1):
    nqt = nq // 128   # q tiles
    nkt = nk // 128   # k tiles
    ndc = nd // 128   # d chunks (contraction)
    nkb = nk // 512   # k blocks (S free dim)
    nqb = nq // 512   # q blocks

    cdt = BF16 if ctx_bf16 else F32R  # dtype of the context-matmul operands

    nc = bacc.Bacc(None, target_bir_lowering=False, debug=False)
    q_p = nc.declare_dram_parameter("q", [bpc, nq, nd], F32, isOutput=False)
    k_p = nc.declare_dram_parameter("k", [bpc, nk, nd], F32, isOutput=False)
    v_p = nc.declare_dram_parameter("v", [bpc, nk, nd], F32, isOutput=False)
    mrow_p = nc.declare_dram_parameter("mrow", [bpc, nk], F32, isOutput=False)
    w_p = nc.declare_dram_parameter("w", [bpc, nq, nk], F32, isOutput=True)
    ctx_p = nc.declare_dram_parameter("ctx", [bpc, nq, nd], F32, isOutput=True)

    with tile.TileContext(nc) as tc:
        with (
            tc.tile_pool(name="const", bufs=1) as constp,
            tc.tile_pool(name="big", bufs=1) as bigp,
            tc.tile_pool(name="work", bufs=2) as workp,
            tc.tile_pool(name="wpool", bufs=3) as wpool,
            tc.tile_pool(name="small", bufs=4) as smallp,
            tc.tile_pool(name="trp", bufs=2, space="PSUM") as trp,
            tc.tile_pool(name="sps", bufs=4, space="PSUM") as spsp,
            tc.tile_pool(name="ctxps", bufs=2, space="PSUM") as ctxpsp,
        ):
            ident = constp.tile([128, 128], F32)
            _make_identity(nc, ident)

            rep_ctx = (
                tc.For_i(0, repeat, 1) if repeat > 1 else contextlib.nullcontext()
            )
            with rep_ctx:
                for b in range(bpc):
                    mrow = workp.tile([128, nk], F32, tag="mrow")
                    nc.sync.dma_start(
                        out=mrow, in_=mrow_p[b, :].partition_broadcast(128)
                    )

                    # K -> Kt [d, k] (f32r), via PE transpose of natural tiles
                    Kt = bigp.tile([128, ndc, nk], F32R, tag="Kt")
                    for kt in range(nkt):
                        k_nat = workp.tile([128, nd], F32, tag="k_nat")
                        nc.sync.dma_start(
                            out=k_nat, in_=k_p[b, kt * 128 : (kt + 1) * 128, :]
                        )
                        for dc in range(ndc):
                            tp = trp.tile([128, 128], F32, tag="tr")
                            nc.tensor.transpose(
                                tp, k_nat[:, dc * 128 : (dc + 1) * 128], ident
                            )
                            nc.vector.tensor_copy(
                                Kt[:, dc, kt * 128 : (kt + 1) * 128], tp
                            )

                    # V (natural [k, d]) -> ctx dtype
                    Vr = bigp.tile([128, nkt, nd], cdt, tag="Vr")
                    for kt in range(nkt):
                        v_nat = workp.tile([128, nd], F32, tag="v_nat")
                        nc.sync.dma_start(
                            out=v_nat, in_=v_p[b, kt * 128 : (kt + 1) * 128, :]
                        )
                        nc.vector.tensor_copy(Vr[:, kt, :], v_nat)

                    for qb in range(nqb):
                        # Q tiles of this q-block -> Qt [d, 512] (f32r)
                        Qt = workp.tile([128, ndc, 512], F32R, tag="Qt")
                        for qtl in range(4):
                            qt = qb * 4 + qtl
                            q_nat = workp.tile([128, nd], F32, tag="q_nat")
                            nc.sync.dma_start(
                                out=q_nat, in_=q_p[b, qt * 128 : (qt + 1) * 128, :]
                            )
                            for dc in range(ndc):
                                tp = trp.tile([128, 128], F32, tag="tr")
                                nc.tensor.transpose(
                                    tp, q_nat[:, dc * 128 : (dc + 1) * 128], ident
                                )
                                nc.vector.tensor_copy(
                                    Qt[:, dc, qtl * 128 : (qtl + 1) * 128], tp
                                )

                        # S pass: exp(S)[q,k] into W staging + rowsum partials
                        wouts, rcs = [], []
                        for qtl in range(4):
                            wout = wpool.tile(
                                [128, nk], F32, tag="wout", name=f"wout{qtl}"
                            )
                            sums = smallp.tile([128, nkb], F32, tag="sums")
                            sp_tiles = [
                                spsp.tile([128, 512], F32, tag="sps", name=f"sps{i}")
                                for i in range(nkb)
                            ]
                            for dc in range(ndc):
                                for kb in range(nkb):
                                    nc.tensor.matmul(
                                        sp_tiles[kb],
                                        Qt[:, dc, qtl * 128 : (qtl + 1) * 128],
                                        Kt[:, dc, kb * 512 : (kb + 1) * 512],
                                        start=(dc == 0),
                                        stop=(dc == ndc - 1),
                                    )
                            for kb in range(nkb):
                                nc.vector.tensor_add(
                                    sp_tiles[kb],
                                    sp_tiles[kb],
                                    mrow[:, kb * 512 : (kb + 1) * 512],
                                )
                                nc.scalar.activation(
                                    wout[:, kb * 512 : (kb + 1) * 512],
                                    sp_tiles[kb],
                                    AF.Exp,
                                    accum_out=sums[:, kb : kb + 1],
                                )
                            rs = smallp.tile([128, 1], F32, tag="rs")
                            nc.vector.reduce_sum(rs, sums, axis=mybir.AxisListType.X)
                            rc = smallp.tile([128, 1], F32, tag="rc")
                            nc.vector.reciprocal(rc, rs)
                            wouts.append(wout)
                            rcs.append(rc)

                        # expSt via PE transpose of exp(S) -- one long burst to
                        # keep PE phases homogeneous (HAM friendliness)
                        expSt = bigp.tile([128, nkt, 512], cdt, tag="expSt")
                        for qtl in range(4):
                            wout = wouts[qtl]
                            for kt in range(nkt):
                                tp = trp.tile([128, 128], F32, tag="tr")
                                nc.tensor.transpose(
                                    tp, wout[:, kt * 128 : (kt + 1) * 128], ident
                                )
                                nc.vector.tensor_copy(
                                    expSt[:, kt, qtl * 128 : (qtl + 1) * 128], tp
                                )
                        # W output: normalize in place, DMA out
                        for qtl in range(4):
                            qt = qb * 4 + qtl
                            wout = wouts[qtl]
                            nc.scalar.activation(wout, wout, AF.Copy, scale=rcs[qtl])
                            nc.sync.dma_start(
                                out=w_p[b, qt * 128 : (qt + 1) * 128, :], in_=wout
                            )
                        # context matmuls -- one long MM burst
                        for qtl in range(4):
                            qt = qb * 4 + qtl
                            cps = ctxpsp.tile([128, nd], F32, tag="cps")
                            for kt in range(nkt):
                                nc.tensor.matmul(
                                    cps,
                                    expSt[:, kt, qtl * 128 : (qtl + 1) * 128],
                                    Vr[:, kt, :],
                                    start=(kt == 0),
                                    stop=(kt == nkt - 1),
                                )
                            ctx_sb = workp.tile([128, nd], F32, tag="ctx_sb")
                            nc.vector.tensor_scalar_mul(ctx_sb, cps, rcs[qtl])
                            nc.sync.dma_start(
                                out=ctx_p[b, qt * 128 : (qt + 1) * 128, :], in_=ctx_sb
                            )

    nc.finalize()
    return nc


_NC_CACHE = {}


def _get_nc():
    if "nc" not in _NC_CACHE:
        _NC_CACHE["nc"] = build_nc()
    return _NC_CACHE["nc"]


def make_inputs(queries, keys, values, valid_lens):
    """Host-side prep: prescale Q, build additive mask, shard across cores."""
    scale = np.float32(1.0 / np.sqrt(np.float32(queries.shape[-1])))
    q = (np.asarray(queries, dtype=np.float32) * scale).astype(np.float32)
    k = np.ascontiguousarray(np.asarray(keys, dtype=np.float32))
    v = np.ascontiguousarray(np.asarray(values, dtype=np.float32))
    vl = np.asarray(valid_lens).astype(np.int64)
    nk = k.shape[1]
    key_pos = np.arange(nk)
    mrow = np.where(key_pos[None, :] < vl[:, None], 0.0, MASK_NEG).astype(np.float32)

    in_maps = []
    for c in range(NCORES):
        sl = slice(c * BPC, (c + 1) * BPC)
        in_maps.append(
            {
                "q": np.ascontiguousarray(q[sl]),
                "k": np.ascontiguousarray(k[sl]),
                "v": np.ascontiguousarray(v[sl]),
                "mrow": np.ascontiguousarray(mrow[sl]),
            }
        )
    return in_maps


def kernel(queries, keys, values, valid_lens):
    in_maps = make_inputs(queries, keys, values, valid_lens)
    nc = _get_nc()
    res = run_bass_kernel_spmd(nc, in_maps, list(range(NCORES)))
    ctx = np.concatenate([res.results[c]["ctx"] for c in range(NCORES)], axis=0)
    w = np.concatenate([res.results[c]["w"] for c in range(NCORES)], axis=0)
    return ctx, w
